# revision 51
# baseline (speedup 1.0000x reference)
"""Trainium2 Bass kernel for a GRU actor-critic network.

Reference computation (per batch row b of B=4096):
    x_gates[t] = features[b,t,:] @ w_ih.T + b_ih            # [T, 3H]
    GRU scan over T=64 steps (torch gate order r, z, n):
        r = sigmoid(xr + hr + b_ihr + b_hhr)
        z = sigmoid(xz + hz + b_ihz + b_hhz)
        n = tanh(xn + b_ihn + r * (hn + b_hhn))
        h = (1-z)*n + z*h
    out = leaky_relu(h_last)
    pi  = leaky_relu(out @ w_pi.T + b_pi)                   # [B, 64]
    vf  = leaky_relu(out @ w_vf.T + b_vf)                   # [B, 64]

Strategy: pure data parallel over 8 cores (512 batch rows each).  On-chip
layout is [gate/hidden on partitions, batch on free] so the recurrent
matmul contracts over the partition dim without per-step transposes.
2 independent batch chains of 256 rows, staggered in time.

v7 design notes (dispatch-count rewrite of the v3 baseline):
  * Gate-major PSUM banks: each of the 8 banks holds ONE gate block
    (r0,r1,z0,z1,n0,n1,hn0,hn1) for the whole 512-row board, so the input
    GEMM for a block is a single N=512 matmul (6/step instead of 12) and
    every bank has a uniform per-partition bias row, premerged by its
    K=2 fp8-DoubleRow opener matmul (which also performs the mandatory
    bank zeroing).
  * Recurrent GEMMs in fp8 e4m3 DoubleRow (K=256 per instruction); h is
    kept in bf16 for the elementwise update and written a second time as
    fp8 for the next matmul.
  * Per chain: one merged sigmoid over [r|z] (strided over 4 banks), one
    tanh; tr/u on DVE; h' on Pool; d/m/h8 on DVE.
  * ~78 instruction dispatches per step (vs 82 for v3, with shorter
    PE head-of-line chains); on current hardware the kernel is bound by
    a mix of per-instruction dispatch overhead and the serial chain.
"""

import os
import sys

import numpy as np
import ml_dtypes

if "/opt/trn_rl_repo" not in sys.path:
    sys.path.insert(0, "/opt/trn_rl_repo")

P = 128          # partitions
H = 256          # GRU hidden
F = 128          # feature dim
T = 64           # sequence length
OUT = 64         # head dim
B = 4096         # full batch
NCORES = 8
BLOC = B // NCORES   # 512 rows per core
CH = 2               # independent batch chains per core
BC = BLOC // CH      # 256 rows per chain
NEG_SLOPE = 0.01

_cache = {}


def build_nc(t_steps=T, loop_n=1):
    import concourse.bass as bass
    import concourse.tile as tile
    from concourse import bacc, mybir

    f32 = mybir.dt.float32
    bf16 = mybir.dt.bfloat16
    fp8 = mybir.dt.float8e4
    AF = mybir.ActivationFunctionType
    OP = mybir.AluOpType
    PSUM = bass.MemorySpace.PSUM
    DR = mybir.MatmulPerfMode.DoubleRow

    nc = bacc.Bacc("TRN2", target_bir_lowering=False, debug=False)

    # features, bf16: [f, t, b] = feat[b, t, f]
    featT = nc.declare_dram_parameter("featT", [F, T, BLOC], bf16, isOutput=False)
    # input weights, bf16 lhsT: [f, m] = w_ih[m, f]
    w_ihT = nc.declare_dram_parameter("w_ihT", [P, 6 * P], bf16, isOutput=False)
    # recurrent weights, fp8 k-packed: [p, s, m] = w_hh[m, s*128+p]
    w_hh8 = nc.declare_dram_parameter("w_hh8", [P, 2, 6 * P], fp8, isOutput=False)
    # per-bank bias rows (gate-major banks have one bias vector each):
    # (r0, r1, z0, z1, ihn0, ihn1, hhn0, hhn1)
    biasg = nc.declare_dram_parameter("biasg", [8, 2, P], fp8, isOutput=False)
    ones8 = nc.declare_dram_parameter("ones8", [1, 2, BLOC], fp8, isOutput=False)
    w_piT = nc.declare_dram_parameter("w_piT", [P, 2, OUT], bf16, isOutput=False)
    w_vfT = nc.declare_dram_parameter("w_vfT", [P, 2, OUT], bf16, isOutput=False)
    b_pv = nc.declare_dram_parameter("b_pv", [P, 2, OUT], f32, isOutput=False)
    out_pi = nc.declare_dram_parameter("pi", [BLOC, OUT], f32, isOutput=True)
    out_vf = nc.declare_dram_parameter("vf", [BLOC, OUT], f32, isOutput=True)

    with tile.TileContext(nc) as tc:
        from contextlib import ExitStack

        ctx = ExitStack()
        with ctx:
            singles = ctx.enter_context(tc.tile_pool(name="singles", bufs=1))
            hsb = ctx.enter_context(tc.tile_pool(name="hsb", bufs=4))

            # ---- weights / biases ----
            sb_wih = singles.tile([P, 6 * P], bf16)
            nc.sync.dma_start(out=sb_wih, in_=w_ihT[:, :])
            sb_whh8 = singles.tile([P, 2, 6 * P], fp8)
            nc.sync.dma_start(out=sb_whh8, in_=w_hh8[:, :, :])
            sb_bg = []
            for k in range(8):
                tbg = singles.tile([1, 2, P], fp8, tag=f"bg{k}")
                nc.sync.dma_start(out=tbg, in_=biasg[k : k + 1, :, :])
                sb_bg.append(tbg)
            sb_on = singles.tile([1, 2, BLOC], fp8)
            nc.sync.dma_start(out=sb_on, in_=ones8[:, :, :])
            sb_wpi = singles.tile([P, 2, OUT], bf16)
            nc.sync.dma_start(out=sb_wpi, in_=w_piT[:, :, :])
            sb_wvf = singles.tile([P, 2, OUT], bf16)
            nc.sync.dma_start(out=sb_wvf, in_=w_vfT[:, :, :])
            sb_bpv = singles.tile([P, 2, OUT], f32)
            nc.sync.dma_start(out=sb_bpv, in_=b_pv[:, :, :])

            # ---- features: host-prepped bf16 [f, t, b], chunked DMA ----
            fT = singles.tile([P, t_steps, BLOC], bf16)
            n_chunk_t = min(8, t_steps)
            for c in range(t_steps // n_chunk_t):
                sl = slice(c * n_chunk_t, (c + 1) * n_chunk_t)
                nc.sync.dma_start(
                    out=fT[:, sl, :],
                    in_=featT[:, sl, :],
                )

            # ---- recurrence ----
            loop_ctx = ExitStack()
            if loop_n > 1:
                loop_ctx.enter_context(tc.For_i(0, loop_n, 1))
            with loop_ctx, ExitStack() as rctx:
                # Gate-major PSUM banks: rzq [P, 4 banks(r0,r1,z0,z1), BLOC],
                # xaq [P, 2(n0,n1), BLOC], xbq [P, 2(hn0,hn1), BLOC].
                # Each bank holds one gate block for BOTH chains, so the
                # input GEMM for a block is a single N=512 matmul and every
                # bank has a uniform bias row (premerged by its opener).
                ps_rz = rctx.enter_context(
                    tc.tile_pool(name="ps_rz", bufs=1, space=PSUM)
                )
                ps_xa = rctx.enter_context(
                    tc.tile_pool(name="ps_xa", bufs=1, space=PSUM)
                )
                ps_xb = rctx.enter_context(
                    tc.tile_pool(name="ps_xb", bufs=1, space=PSUM)
                )
                gates = [
                    rctx.enter_context(tc.tile_pool(name=f"gates{c}", bufs=2))
                    for c in range(CH)
                ]
                hpool = [
                    rctx.enter_context(tc.tile_pool(name=f"hpool{c}", bufs=2))
                    for c in range(CH)
                ]
                h8pool = [
                    rctx.enter_context(tc.tile_pool(name=f"h8pool{c}", bufs=2))
                    for c in range(CH)
                ]

                h_prev = []
                h8_prev = []
                for c in range(CH):
                    h0 = hpool[c].tile([P, 2 * BC], bf16, tag="h")
                    nc.vector.memset(h0, 0.0)
                    h_prev.append(h0)
                    h80 = h8pool[c].tile([P, 2, BC], fp8, tag="h8")
                    nc.gpsimd.memset(h80, 0.0)
                    h8_prev.append(h80)

                for t in range(t_steps):
                    rzq = ps_rz.tile([P, 4, BLOC], f32, tag="rzq", name="rzq")
                    xaq = ps_xa.tile([P, 2, BLOC], f32, tag="xaq", name="xaq")
                    xbq = ps_xb.tile([P, 2, BLOC], f32, tag="xbq", name="xbq")
                    sig_t, trt_t, u_t, nt_t = [], [], [], []
                    for c in range(CH):
                        sig_t.append(gates[c].tile([P, 4, BC], bf16, tag="sig", name="sig"))
                        trt_t.append(gates[c].tile([P, 2 * BC], bf16, tag="tr", name="tr"))
                        u_t.append(gates[c].tile([P, 2 * BC], bf16, tag="u", name="u"))
                        nt_t.append(gates[c].tile([P, 2 * BC], bf16, tag="nt", name="nt"))

                    # --- bank openers premerge the per-bank bias rows ---
                    for k in range(4):
                        nc.tensor.matmul(
                            rzq[:, k, :], sb_bg[k], sb_on,
                            start=True, stop=False, perf_mode=DR,
                        )
                    for j in range(2):
                        nc.tensor.matmul(
                            xaq[:, j, :], sb_bg[4 + j], sb_on,
                            start=True, stop=False, perf_mode=DR,
                        )
                        nc.tensor.matmul(
                            xbq[:, j, :], sb_bg[6 + j], sb_on,
                            start=True, stop=False, perf_mode=DR,
                        )
                    # --- input GEMMs: one N=512 matmul per gate block ---
                    f_t = fT[:, t, :]
                    for k in range(4):
                        nc.tensor.matmul(
                            rzq[:, k, :], sb_wih[:, k * P : (k + 1) * P], f_t,
                            start=False, stop=False,
                        )
                    for j in range(2):
                        nc.tensor.matmul(
                            xaq[:, j, :], sb_wih[:, (4 + j) * P : (5 + j) * P], f_t,
                            start=False, stop=True,
                        )
                    # --- recurrent GEMMs per chain (close rz and xb) ---
                    for c in range(CH):
                        csl = slice(c * BC, (c + 1) * BC)
                        for k in range(4):
                            nc.tensor.matmul(
                                rzq[:, k, csl],
                                sb_whh8[:, :, k * P : (k + 1) * P],
                                h8_prev[c][:, :, :],
                                start=False, stop=(c == CH - 1), perf_mode=DR,
                            )
                        for j in range(2):
                            nc.tensor.matmul(
                                xbq[:, j, csl],
                                sb_whh8[:, :, (4 + j) * P : (5 + j) * P],
                                h8_prev[c][:, :, :],
                                start=False, stop=(c == CH - 1), perf_mode=DR,
                            )

                    # --- sig = [r|z] per chain (strided over 4 banks) ---
                    for c in range(CH):
                        csl = slice(c * BC, (c + 1) * BC)
                        nc.scalar.activation(sig_t[c], rzq[:, :, csl], AF.Sigmoid)
                    # --- tr = (hn + b_hhn) * r; u = xn + b_ihn + tr (DVE) ---
                    for c in range(CH):
                        csl = slice(c * BC, (c + 1) * BC)
                        nc.vector.tensor_tensor(
                            trt_t[c].rearrange("p (j c) -> p j c", j=2),
                            xbq[:, :, csl],
                            sig_t[c][:, 0:2, :],
                            OP.mult,
                        )
                        nc.vector.tensor_tensor(
                            u_t[c].rearrange("p (j c) -> p j c", j=2),
                            xaq[:, :, csl],
                            trt_t[c].rearrange("p (j c) -> p j c", j=2),
                            OP.add,
                        )
                    # --- n = tanh(u) ---
                    for c in range(CH):
                        nc.scalar.activation(nt_t[c], u_t[c], AF.Tanh)
                    # --- h' = n + z*(h - n); h8 same in fp8 ---
                    for c in range(CH):
                        hp = h_prev[c]
                        nt = nt_t[c]
                        d = gates[c].tile([P, 2 * BC], bf16, tag="d", name="d")
                        m = gates[c].tile([P, 2 * BC], bf16, tag="m", name="m")
                        nc.vector.tensor_tensor(d, hp, nt, OP.subtract)
                        nc.vector.tensor_tensor(
                            m,
                            sig_t[c][:, 2:4, :].rearrange("p j c -> p (j c)"),
                            d,
                            OP.mult,
                        )
                        h8_new = h8pool[c].tile([P, 2, BC], fp8, tag="h8")
                        nc.vector.tensor_tensor(
                            h8_new.rearrange("p j c -> p (j c)"), nt, m,
                            OP.add,
                        )
                        h_new = hpool[c].tile([P, 2 * BC], bf16, tag="h")
                        nc.gpsimd.tensor_tensor(h_new, nt, m, OP.add)
                        h_prev[c] = h_new
                        h8_prev[c] = h8_new

            # ---- heads ----
            with ExitStack() as hctx:
                pshead = hctx.enter_context(
                    tc.tile_pool(name="pshead", bufs=4, space=PSUM)
                )
                for c in range(CH):
                    lt = singles.tile([P, 2 * BC], bf16, tag=f"lr{c}")
                    nc.vector.scalar_tensor_tensor(
                        out=lt,
                        in0=h_prev[c],
                        scalar=NEG_SLOPE,
                        in1=h_prev[c],
                        op0=OP.mult,
                        op1=OP.max,
                    )
                    for head, (wT, out_dram) in enumerate(
                        [(sb_wpi, out_pi), (sb_wvf, out_vf)]
                    ):
                        for mm in range(BC // P):
                            pp = pshead.tile([P, OUT], f32, tag="pp")
                            for j in range(2):
                                nc.tensor.matmul(
                                    pp,
                                    lt[:, j * BC + mm * P : j * BC + (mm + 1) * P],
                                    wT[:, j, :],
                                    start=(j == 0),
                                    stop=(j == 1),
                                )
                            q = hsb.tile([P, OUT], f32, tag="q")
                            nc.vector.tensor_tensor(
                                q, pp, sb_bpv[:, head, :], OP.add
                            )
                            o = hsb.tile([P, OUT], f32, tag="o")
                            nc.vector.scalar_tensor_tensor(
                                out=o,
                                in0=q,
                                scalar=NEG_SLOPE,
                                in1=q,
                                op0=OP.mult,
                                op1=OP.max,
                            )
                            r0 = c * BC + mm * P
                            nc.scalar.dma_start(
                                out=out_dram[r0 : r0 + P, :], in_=o
                            )

    return nc


def prep_inputs(inputs):
    """Host-side prep: shard features, build weight/bias layouts."""
    bf = ml_dtypes.bfloat16
    e4 = ml_dtypes.float8_e4m3
    feat = np.asarray(inputs["features"], np.float32).reshape(B, T, F)
    w_ih = np.asarray(inputs["w_ih"], np.float32)
    w_hh = np.asarray(inputs["w_hh"], np.float32)
    b_ih = np.asarray(inputs["b_ih"], np.float32)
    b_hh = np.asarray(inputs["b_hh"], np.float32)
    w_pi = np.asarray(inputs["w_pi"], np.float32)
    b_pi = np.asarray(inputs["b_pi"], np.float32)
    w_vf = np.asarray(inputs["w_vf"], np.float32)
    b_vf = np.asarray(inputs["b_vf"], np.float32)

    w_ihT = np.ascontiguousarray(w_ih.T).astype(bf)                       # [128, 768]
    w_hh8 = np.ascontiguousarray(
        w_hh.T.reshape(2, P, 6 * P).transpose(1, 0, 2)
    ).astype(e4)                                                          # [128, 2, 768]
    b_c = b_ih + b_hh
    # per-bank bias rows; second k-subtile is zeros.
    biasg = np.zeros((8, 2, P), np.float32)
    biasg[:, 0, :] = [
        b_c[0:128], b_c[128:256], b_c[256:384], b_c[384:512],
        b_ih[512:640], b_ih[640:768], b_hh[512:640], b_hh[640:768],
    ]
    biasg = biasg.astype(e4)
    ones8 = np.zeros((1, 2, BLOC), np.float32)
    ones8[0, 0, :] = 1.0
    ones8 = ones8.astype(e4)

    w_piT = np.ascontiguousarray(
        w_pi.T.reshape(2, P, OUT).transpose(1, 0, 2)
    ).astype(bf)
    w_vfT = np.ascontiguousarray(
        w_vf.T.reshape(2, P, OUT).transpose(1, 0, 2)
    ).astype(bf)
    b_pv = np.ascontiguousarray(
        np.broadcast_to(np.stack([b_pi, b_vf], axis=0), (P, 2, OUT))
    ).astype(np.float32)

    shared = {
        "w_ihT": w_ihT,
        "w_hh8": w_hh8,
        "biasg": biasg,
        "ones8": ones8,
        "w_piT": w_piT,
        "w_vfT": w_vfT,
        "b_pv": b_pv,
    }
    in_maps = []
    for i in range(NCORES):
        m = dict(shared)
        shard = feat[i * BLOC : (i + 1) * BLOC]        # [BLOC, T, F]
        m["featT"] = np.ascontiguousarray(
            shard.transpose(2, 1, 0)
        ).astype(bf)                                    # [F, T, BLOC]
        in_maps.append(m)
    return in_maps


def _get_nc():
    if "nc" not in _cache:
        nc = build_nc()
        nc.finalize()
        _cache["nc"] = nc
    return _cache["nc"]


def _get_runner():
    """Build (once) a cached jitted shard_map executor for the bass program."""
    if "runner" in _cache:
        return _cache["runner"]

    import jax
    from jax.experimental.shard_map import shard_map
    from jax.sharding import Mesh, PartitionSpec
    from concourse import bass2jax, mybir

    nc = _get_nc()
    bass2jax.install_neuronx_cc_hook()

    partition_name = (
        nc.partition_id_tensor.name if nc.partition_id_tensor else None
    )
    in_names, out_names, out_avals, zero_outs = [], [], [], []
    for alloc in nc.m.functions[0].allocations:
        if not isinstance(alloc, mybir.MemoryLocationSet):
            continue
        name = alloc.memorylocations[0].name
        if alloc.kind == "ExternalInput":
            if name != partition_name:
                in_names.append(name)
        elif alloc.kind == "ExternalOutput":
            out_names.append(name)
            shape = tuple(alloc.tensor_shape)
            dtype = mybir.dt.np(alloc.dtype)
            out_avals.append(jax.core.ShapedArray(shape, dtype))
            zero_outs.append(np.zeros(shape, dtype))
    n_params = len(in_names)
    n_outs = len(out_avals)
    all_names = in_names + out_names
    if partition_name is not None:
        all_names = all_names + [partition_name]

    def _body(*args):
        operands = list(args)
        if partition_name is not None:
            operands.append(bass2jax.partition_id_tensor())
        outs = bass2jax._bass_exec_p.bind(
            *operands,
            out_avals=tuple(out_avals),
            in_names=tuple(all_names),
            out_names=tuple(out_names),
            lowering_input_output_aliases=(),
            sim_require_finite=True,
            sim_require_nnan=True,
            nc=nc,
        )
        return tuple(outs)

    donate = tuple(range(n_params, n_params + n_outs))
    devices = jax.devices()[:NCORES]
    mesh = Mesh(np.asarray(devices), ("core",))
    sharded = jax.jit(
        shard_map(
            _body,
            mesh=mesh,
            in_specs=(PartitionSpec("core"),) * (n_params + n_outs),
            out_specs=(PartitionSpec("core"),) * n_outs,
            check_rep=False,
        ),
        donate_argnums=donate,
        keep_unused=True,
    )

    from jax.sharding import NamedSharding

    shard_spec = NamedSharding(mesh, PartitionSpec("core"))
    state = {}

    def run(in_maps, timeit=False):
        key = id(in_maps)
        if state.get("key") != key:
            concat_in = [
                np.concatenate([np.asarray(m[n]) for m in in_maps], axis=0)
                for n in in_names
            ]
            state["dev_in"] = [
                jax.device_put(a, shard_spec) for a in concat_in
            ]
            for a in state["dev_in"]:
                a.block_until_ready()
            state["key"] = key
        concat_zeros = [
            jax.device_put(
                np.zeros((NCORES * z.shape[0], *z.shape[1:]), z.dtype),
                shard_spec,
            )
            for z in zero_outs
        ]
        out_arrs = sharded(*state["dev_in"], *concat_zeros)
        jax.block_until_ready(out_arrs)
        outs = {
            name: np.asarray(out_arrs[i]) for i, name in enumerate(out_names)
        }
        return outs

    _cache["runner"] = run
    return run


def kernel(**inputs):
    run = _get_runner()
    in_maps = prep_inputs(inputs)
    outs = run(in_maps)
    pi = outs["pi"].astype(np.float32)
    vf = outs["vf"].astype(np.float32)
    return pi, vf


def kernel_timed(inputs, iters=10):
    """Returns (pi, vf, per_call_seconds) with device-resident inputs."""
    import time

    run = _get_runner()
    in_maps = prep_inputs(inputs)
    outs = run(in_maps)  # warmup + input upload
    t0 = time.monotonic()
    for _ in range(iters):
        outs = run(in_maps)
    dt = (time.monotonic() - t0) / iters
    pi = outs["pi"].astype(np.float32)
    vf = outs["vf"].astype(np.float32)
    return pi, vf, dt


# revision 55
# speedup vs baseline: 1.0163x; 1.0163x over previous
"""Trainium2 Bass kernel for a GRU actor-critic network.

Reference computation (per batch row b of B=4096):
    x_gates[t] = features[b,t,:] @ w_ih.T + b_ih            # [T, 3H]
    GRU scan over T=64 steps (torch gate order r, z, n):
        r = sigmoid(xr + hr + b_ihr + b_hhr)
        z = sigmoid(xz + hz + b_ihz + b_hhz)
        n = tanh(xn + b_ihn + r * (hn + b_hhn))
        h = (1-z)*n + z*h
    out = leaky_relu(h_last)
    pi  = leaky_relu(out @ w_pi.T + b_pi)                   # [B, 64]
    vf  = leaky_relu(out @ w_vf.T + b_vf)                   # [B, 64]

Strategy: pure data parallel over 8 cores (512 batch rows each).  On-chip
layout is [gate/hidden on partitions, batch on free] so the recurrent
matmul contracts over the partition dim without per-step transposes.
2 independent batch chains of 256 rows, staggered in time.

v7 design notes (dispatch-count rewrite of the v3 baseline):
  * Gate-major PSUM banks: each of the 8 banks holds ONE gate block
    (r0,r1,z0,z1,n0,n1,hn0,hn1) for the whole 512-row board, so the input
    GEMM for a block is a single N=512 matmul (6/step instead of 12) and
    every bank has a uniform per-partition bias row, premerged by its
    K=2 fp8-DoubleRow opener matmul (which also performs the mandatory
    bank zeroing).
  * Recurrent GEMMs in fp8 e4m3 DoubleRow (K=256 per instruction); h is
    kept in bf16 for the elementwise update and written a second time as
    fp8 for the next matmul.
  * Per chain: one merged sigmoid over [r|z] (strided over 4 banks), one
    tanh; tr/u on DVE; h' on Pool; d/m/h8 on DVE.
  * ~78 instruction dispatches per step (vs 82 for v3, with shorter
    PE head-of-line chains); on current hardware the kernel is bound by
    a mix of per-instruction dispatch overhead and the serial chain.
"""

import os
import sys

import numpy as np
import ml_dtypes

if "/opt/trn_rl_repo" not in sys.path:
    sys.path.insert(0, "/opt/trn_rl_repo")

P = 128          # partitions
H = 256          # GRU hidden
F = 128          # feature dim
T = 64           # sequence length
OUT = 64         # head dim
B = 4096         # full batch
NCORES = 8
BLOC = B // NCORES   # 512 rows per core
CH = 2               # independent batch chains per core
BC = BLOC // CH      # 256 rows per chain
NEG_SLOPE = 0.01

_cache = {}


def build_nc(t_steps=T, loop_n=1):
    import concourse.bass as bass
    import concourse.tile as tile
    from concourse import bacc, mybir

    f32 = mybir.dt.float32
    bf16 = mybir.dt.bfloat16
    fp8 = mybir.dt.float8e4
    AF = mybir.ActivationFunctionType
    OP = mybir.AluOpType
    PSUM = bass.MemorySpace.PSUM
    DR = mybir.MatmulPerfMode.DoubleRow

    nc = bacc.Bacc("TRN2", target_bir_lowering=False, debug=False)

    # features, bf16: [f, t, b] = feat[b, t, f]
    featT = nc.declare_dram_parameter("featT", [F, T, BLOC], bf16, isOutput=False)
    # input weights, bf16 lhsT: [f, m] = w_ih[m, f]
    w_ihT = nc.declare_dram_parameter("w_ihT", [P, 6 * P], bf16, isOutput=False)
    # recurrent weights, fp8 k-packed: [p, s, m] = w_hh[m, s*128+p]
    w_hh8 = nc.declare_dram_parameter("w_hh8", [P, 2, 6 * P], fp8, isOutput=False)
    # per-bank bias rows (gate-major banks have one bias vector each):
    # (r0, r1, z0, z1, ihn0, ihn1, hhn0, hhn1)
    biasg = nc.declare_dram_parameter("biasg", [8, 2, P], fp8, isOutput=False)
    ones8 = nc.declare_dram_parameter("ones8", [1, 2, BLOC], fp8, isOutput=False)
    w_piT = nc.declare_dram_parameter("w_piT", [P, 2, OUT], bf16, isOutput=False)
    w_vfT = nc.declare_dram_parameter("w_vfT", [P, 2, OUT], bf16, isOutput=False)
    b_pv = nc.declare_dram_parameter("b_pv", [P, 2, OUT], f32, isOutput=False)
    out_pi = nc.declare_dram_parameter("pi", [BLOC, OUT], f32, isOutput=True)
    out_vf = nc.declare_dram_parameter("vf", [BLOC, OUT], f32, isOutput=True)

    with tile.TileContext(nc) as tc:
        from contextlib import ExitStack

        ctx = ExitStack()
        with ctx:
            singles = ctx.enter_context(tc.tile_pool(name="singles", bufs=1))
            hsb = ctx.enter_context(tc.tile_pool(name="hsb", bufs=4))

            # ---- weights / biases ----
            sb_wih = singles.tile([P, 6 * P], bf16)
            nc.sync.dma_start(out=sb_wih, in_=w_ihT[:, :])
            sb_whh8 = singles.tile([P, 2, 6 * P], fp8)
            nc.sync.dma_start(out=sb_whh8, in_=w_hh8[:, :, :])
            sb_bg = []
            for k in range(8):
                tbg = singles.tile([1, 2, P], fp8, tag=f"bg{k}")
                nc.sync.dma_start(out=tbg, in_=biasg[k : k + 1, :, :])
                sb_bg.append(tbg)
            sb_on = singles.tile([1, 2, BLOC], fp8)
            nc.sync.dma_start(out=sb_on, in_=ones8[:, :, :])
            sb_wpi = singles.tile([P, 2, OUT], bf16)
            nc.sync.dma_start(out=sb_wpi, in_=w_piT[:, :, :])
            sb_wvf = singles.tile([P, 2, OUT], bf16)
            nc.sync.dma_start(out=sb_wvf, in_=w_vfT[:, :, :])
            sb_bpv = singles.tile([P, 2, OUT], f32)
            nc.sync.dma_start(out=sb_bpv, in_=b_pv[:, :, :])

            # ---- features: host-prepped bf16 [f, t, b], chunked DMA ----
            fT = singles.tile([P, t_steps, BLOC], bf16)
            n_chunk_t = min(8, t_steps)
            for c in range(t_steps // n_chunk_t):
                sl = slice(c * n_chunk_t, (c + 1) * n_chunk_t)
                nc.sync.dma_start(
                    out=fT[:, sl, :],
                    in_=featT[:, sl, :],
                )

            # ---- recurrence ----
            loop_ctx = ExitStack()
            if loop_n > 1:
                loop_ctx.enter_context(tc.For_i(0, loop_n, 1))
            with loop_ctx, ExitStack() as rctx:
                # Gate-major PSUM banks: rzq [P, 4 banks(r0,r1,z0,z1), BLOC],
                # xaq [P, 2(n0,n1), BLOC], xbq [P, 2(hn0,hn1), BLOC].
                # Each bank holds one gate block for BOTH chains, so the
                # input GEMM for a block is a single N=512 matmul and every
                # bank has a uniform bias row (premerged by its opener).
                ps_rz = rctx.enter_context(
                    tc.tile_pool(name="ps_rz", bufs=1, space=PSUM)
                )
                ps_xa = rctx.enter_context(
                    tc.tile_pool(name="ps_xa", bufs=1, space=PSUM)
                )
                ps_xb = rctx.enter_context(
                    tc.tile_pool(name="ps_xb", bufs=1, space=PSUM)
                )
                gates = [
                    rctx.enter_context(tc.tile_pool(name=f"gates{c}", bufs=2))
                    for c in range(CH)
                ]
                hpool = [
                    rctx.enter_context(tc.tile_pool(name=f"hpool{c}", bufs=2))
                    for c in range(CH)
                ]
                h8pool = [
                    rctx.enter_context(tc.tile_pool(name=f"h8pool{c}", bufs=2))
                    for c in range(CH)
                ]

                h_prev = []
                h8_prev = []
                for c in range(CH):
                    h0 = hpool[c].tile([P, 2 * BC], bf16, tag="h")
                    nc.vector.memset(h0, 0.0)
                    h_prev.append(h0)
                    h80 = h8pool[c].tile([P, 2, BC], fp8, tag="h8")
                    nc.gpsimd.memset(h80, 0.0)
                    h8_prev.append(h80)

                for t in range(t_steps):
                    rzq = ps_rz.tile([P, 4, BLOC], f32, tag="rzq", name="rzq")
                    xaq = ps_xa.tile([P, 2, BLOC], f32, tag="xaq", name="xaq")
                    xbq = ps_xb.tile([P, 2, BLOC], f32, tag="xbq", name="xbq")
                    sig_t, trt_t, u_t, nt_t = [], [], [], []
                    for c in range(CH):
                        sig_t.append(gates[c].tile([P, 4, BC], bf16, tag="sig", name="sig"))
                        trt_t.append(gates[c].tile([P, 2 * BC], bf16, tag="tr", name="tr"))
                        u_t.append(gates[c].tile([P, 2 * BC], bf16, tag="u", name="u"))
                        nt_t.append(gates[c].tile([P, 2 * BC], bf16, tag="nt", name="nt"))

                    # --- bank openers premerge the per-bank bias rows ---
                    for k in range(4):
                        nc.tensor.matmul(
                            rzq[:, k, :], sb_bg[k], sb_on,
                            start=True, stop=False, perf_mode=DR,
                        )
                    for j in range(2):
                        nc.tensor.matmul(
                            xaq[:, j, :], sb_bg[4 + j], sb_on,
                            start=True, stop=False, perf_mode=DR,
                        )
                        nc.tensor.matmul(
                            xbq[:, j, :], sb_bg[6 + j], sb_on,
                            start=True, stop=False, perf_mode=DR,
                        )
                    # --- input GEMMs: one N=512 matmul per gate block ---
                    f_t = fT[:, t, :]
                    for k in range(4):
                        nc.tensor.matmul(
                            rzq[:, k, :], sb_wih[:, k * P : (k + 1) * P], f_t,
                            start=False, stop=False,
                        )
                    for j in range(2):
                        nc.tensor.matmul(
                            xaq[:, j, :], sb_wih[:, (4 + j) * P : (5 + j) * P], f_t,
                            start=False, stop=True,
                        )
                    # --- recurrent GEMMs per chain (close rz and xb) ---
                    # Chain 0's recs close each bank's group; later chains
                    # accumulate with skip_group_check (plain accumulation,
                    # correct on HW) so sigma/tr of chain c only depend on
                    # chain c's own writes -- the chains stay staggered
                    # instead of phase-locking on the shared banks.
                    for c in range(CH):
                        csl = slice(c * BC, (c + 1) * BC)
                        for k in range(4):
                            nc.tensor.matmul(
                                rzq[:, k, csl],
                                sb_whh8[:, :, k * P : (k + 1) * P],
                                h8_prev[c][:, :, :],
                                start=False, stop=(c == 0), perf_mode=DR,
                                skip_group_check=(c > 0),
                            )
                        for j in range(2):
                            nc.tensor.matmul(
                                xbq[:, j, csl],
                                sb_whh8[:, :, (4 + j) * P : (5 + j) * P],
                                h8_prev[c][:, :, :],
                                start=False, stop=(c == 0), perf_mode=DR,
                                skip_group_check=(c > 0),
                            )

                    # --- sig = [r|z] per chain (strided over 4 banks) ---
                    for c in range(CH):
                        csl = slice(c * BC, (c + 1) * BC)
                        nc.scalar.activation(sig_t[c], rzq[:, :, csl], AF.Sigmoid)
                    # --- tr = (hn + b_hhn) * r; u = xn + b_ihn + tr (DVE) ---
                    for c in range(CH):
                        csl = slice(c * BC, (c + 1) * BC)
                        nc.vector.tensor_tensor(
                            trt_t[c].rearrange("p (j c) -> p j c", j=2),
                            xbq[:, :, csl],
                            sig_t[c][:, 0:2, :],
                            OP.mult,
                        )
                        nc.vector.tensor_tensor(
                            u_t[c].rearrange("p (j c) -> p j c", j=2),
                            xaq[:, :, csl],
                            trt_t[c].rearrange("p (j c) -> p j c", j=2),
                            OP.add,
                        )
                    # --- n = tanh(u) ---
                    for c in range(CH):
                        nc.scalar.activation(nt_t[c], u_t[c], AF.Tanh)
                    # --- h' = n + z*(h - n); h8 same in fp8 ---
                    for c in range(CH):
                        hp = h_prev[c]
                        nt = nt_t[c]
                        d = gates[c].tile([P, 2 * BC], bf16, tag="d", name="d")
                        m = gates[c].tile([P, 2 * BC], bf16, tag="m", name="m")
                        nc.vector.tensor_tensor(d, hp, nt, OP.subtract)
                        nc.vector.tensor_tensor(
                            m,
                            sig_t[c][:, 2:4, :].rearrange("p j c -> p (j c)"),
                            d,
                            OP.mult,
                        )
                        h8_new = h8pool[c].tile([P, 2, BC], fp8, tag="h8")
                        nc.vector.tensor_tensor(
                            h8_new.rearrange("p j c -> p (j c)"), nt, m,
                            OP.add,
                        )
                        h_new = hpool[c].tile([P, 2 * BC], bf16, tag="h")
                        nc.gpsimd.tensor_tensor(h_new, nt, m, OP.add)
                        h_prev[c] = h_new
                        h8_prev[c] = h8_new

            # ---- heads ----
            with ExitStack() as hctx:
                pshead = hctx.enter_context(
                    tc.tile_pool(name="pshead", bufs=4, space=PSUM)
                )
                for c in range(CH):
                    lt = singles.tile([P, 2 * BC], bf16, tag=f"lr{c}")
                    nc.vector.scalar_tensor_tensor(
                        out=lt,
                        in0=h_prev[c],
                        scalar=NEG_SLOPE,
                        in1=h_prev[c],
                        op0=OP.mult,
                        op1=OP.max,
                    )
                    for head, (wT, out_dram) in enumerate(
                        [(sb_wpi, out_pi), (sb_wvf, out_vf)]
                    ):
                        for mm in range(BC // P):
                            pp = pshead.tile([P, OUT], f32, tag="pp")
                            for j in range(2):
                                nc.tensor.matmul(
                                    pp,
                                    lt[:, j * BC + mm * P : j * BC + (mm + 1) * P],
                                    wT[:, j, :],
                                    start=(j == 0),
                                    stop=(j == 1),
                                )
                            q = hsb.tile([P, OUT], f32, tag="q")
                            nc.vector.tensor_tensor(
                                q, pp, sb_bpv[:, head, :], OP.add
                            )
                            o = hsb.tile([P, OUT], f32, tag="o")
                            nc.vector.scalar_tensor_tensor(
                                out=o,
                                in0=q,
                                scalar=NEG_SLOPE,
                                in1=q,
                                op0=OP.mult,
                                op1=OP.max,
                            )
                            r0 = c * BC + mm * P
                            nc.scalar.dma_start(
                                out=out_dram[r0 : r0 + P, :], in_=o
                            )

    return nc


def prep_inputs(inputs):
    """Host-side prep: shard features, build weight/bias layouts."""
    bf = ml_dtypes.bfloat16
    e4 = ml_dtypes.float8_e4m3
    feat = np.asarray(inputs["features"], np.float32).reshape(B, T, F)
    w_ih = np.asarray(inputs["w_ih"], np.float32)
    w_hh = np.asarray(inputs["w_hh"], np.float32)
    b_ih = np.asarray(inputs["b_ih"], np.float32)
    b_hh = np.asarray(inputs["b_hh"], np.float32)
    w_pi = np.asarray(inputs["w_pi"], np.float32)
    b_pi = np.asarray(inputs["b_pi"], np.float32)
    w_vf = np.asarray(inputs["w_vf"], np.float32)
    b_vf = np.asarray(inputs["b_vf"], np.float32)

    w_ihT = np.ascontiguousarray(w_ih.T).astype(bf)                       # [128, 768]
    w_hh8 = np.ascontiguousarray(
        w_hh.T.reshape(2, P, 6 * P).transpose(1, 0, 2)
    ).astype(e4)                                                          # [128, 2, 768]
    b_c = b_ih + b_hh
    # per-bank bias rows; second k-subtile is zeros.
    biasg = np.zeros((8, 2, P), np.float32)
    biasg[:, 0, :] = [
        b_c[0:128], b_c[128:256], b_c[256:384], b_c[384:512],
        b_ih[512:640], b_ih[640:768], b_hh[512:640], b_hh[640:768],
    ]
    biasg = biasg.astype(e4)
    ones8 = np.zeros((1, 2, BLOC), np.float32)
    ones8[0, 0, :] = 1.0
    ones8 = ones8.astype(e4)

    w_piT = np.ascontiguousarray(
        w_pi.T.reshape(2, P, OUT).transpose(1, 0, 2)
    ).astype(bf)
    w_vfT = np.ascontiguousarray(
        w_vf.T.reshape(2, P, OUT).transpose(1, 0, 2)
    ).astype(bf)
    b_pv = np.ascontiguousarray(
        np.broadcast_to(np.stack([b_pi, b_vf], axis=0), (P, 2, OUT))
    ).astype(np.float32)

    shared = {
        "w_ihT": w_ihT,
        "w_hh8": w_hh8,
        "biasg": biasg,
        "ones8": ones8,
        "w_piT": w_piT,
        "w_vfT": w_vfT,
        "b_pv": b_pv,
    }
    in_maps = []
    for i in range(NCORES):
        m = dict(shared)
        shard = feat[i * BLOC : (i + 1) * BLOC]        # [BLOC, T, F]
        m["featT"] = np.ascontiguousarray(
            shard.transpose(2, 1, 0)
        ).astype(bf)                                    # [F, T, BLOC]
        in_maps.append(m)
    return in_maps


def _get_nc():
    if "nc" not in _cache:
        nc = build_nc()
        nc.finalize()
        _cache["nc"] = nc
    return _cache["nc"]


def _get_runner():
    """Build (once) a cached jitted shard_map executor for the bass program."""
    if "runner" in _cache:
        return _cache["runner"]

    import jax
    from jax.experimental.shard_map import shard_map
    from jax.sharding import Mesh, PartitionSpec
    from concourse import bass2jax, mybir

    nc = _get_nc()
    bass2jax.install_neuronx_cc_hook()

    partition_name = (
        nc.partition_id_tensor.name if nc.partition_id_tensor else None
    )
    in_names, out_names, out_avals, zero_outs = [], [], [], []
    for alloc in nc.m.functions[0].allocations:
        if not isinstance(alloc, mybir.MemoryLocationSet):
            continue
        name = alloc.memorylocations[0].name
        if alloc.kind == "ExternalInput":
            if name != partition_name:
                in_names.append(name)
        elif alloc.kind == "ExternalOutput":
            out_names.append(name)
            shape = tuple(alloc.tensor_shape)
            dtype = mybir.dt.np(alloc.dtype)
            out_avals.append(jax.core.ShapedArray(shape, dtype))
            zero_outs.append(np.zeros(shape, dtype))
    n_params = len(in_names)
    n_outs = len(out_avals)
    all_names = in_names + out_names
    if partition_name is not None:
        all_names = all_names + [partition_name]

    def _body(*args):
        operands = list(args)
        if partition_name is not None:
            operands.append(bass2jax.partition_id_tensor())
        outs = bass2jax._bass_exec_p.bind(
            *operands,
            out_avals=tuple(out_avals),
            in_names=tuple(all_names),
            out_names=tuple(out_names),
            lowering_input_output_aliases=(),
            sim_require_finite=True,
            sim_require_nnan=True,
            nc=nc,
        )
        return tuple(outs)

    donate = tuple(range(n_params, n_params + n_outs))
    devices = jax.devices()[:NCORES]
    mesh = Mesh(np.asarray(devices), ("core",))
    sharded = jax.jit(
        shard_map(
            _body,
            mesh=mesh,
            in_specs=(PartitionSpec("core"),) * (n_params + n_outs),
            out_specs=(PartitionSpec("core"),) * n_outs,
            check_rep=False,
        ),
        donate_argnums=donate,
        keep_unused=True,
    )

    from jax.sharding import NamedSharding

    shard_spec = NamedSharding(mesh, PartitionSpec("core"))
    state = {}

    def run(in_maps, timeit=False):
        key = id(in_maps)
        if state.get("key") != key:
            concat_in = [
                np.concatenate([np.asarray(m[n]) for m in in_maps], axis=0)
                for n in in_names
            ]
            state["dev_in"] = [
                jax.device_put(a, shard_spec) for a in concat_in
            ]
            for a in state["dev_in"]:
                a.block_until_ready()
            state["key"] = key
        concat_zeros = [
            jax.device_put(
                np.zeros((NCORES * z.shape[0], *z.shape[1:]), z.dtype),
                shard_spec,
            )
            for z in zero_outs
        ]
        out_arrs = sharded(*state["dev_in"], *concat_zeros)
        jax.block_until_ready(out_arrs)
        outs = {
            name: np.asarray(out_arrs[i]) for i, name in enumerate(out_names)
        }
        return outs

    _cache["runner"] = run
    return run


def kernel(**inputs):
    run = _get_runner()
    in_maps = prep_inputs(inputs)
    outs = run(in_maps)
    pi = outs["pi"].astype(np.float32)
    vf = outs["vf"].astype(np.float32)
    return pi, vf


def kernel_timed(inputs, iters=10):
    """Returns (pi, vf, per_call_seconds) with device-resident inputs."""
    import time

    run = _get_runner()
    in_maps = prep_inputs(inputs)
    outs = run(in_maps)  # warmup + input upload
    t0 = time.monotonic()
    for _ in range(iters):
        outs = run(in_maps)
    dt = (time.monotonic() - t0) / iters
    pi = outs["pi"].astype(np.float32)
    vf = outs["vf"].astype(np.float32)
    return pi, vf, dt
